# revision 3
# baseline (speedup 1.0000x reference)
"""ProGen attention (B=2, S=2048, E=2048, H=16, DH=128, partial RoPE 32) on 8 trn2 cores.

Sharding: tensor-parallel over heads — core c owns heads {2c, 2c+1} for both
batches. Per core: QKV projection for its heads (full tokens), causal
flash-style attention (embarrassingly parallel over (b,h)), then an AllToAll
exchanges head-slices for token-slices so each core runs the output
projection for 512 tokens with the full E dimension. Host only permutes
weights in, and concatenates token-slices out.

All matmuls run as float32r (TF32-like, 1 cycle/row at free-dim >= 256).
"""

import sys

sys.path.insert(0, "/opt/trn_rl_repo")

import numpy as np

B = 2
S = 2048
E = 2048
H = 16
DH = 128
ROT = 32
MP = 8
P = 128
T = B * S            # 4096 tokens, t = b*S + s
NCORE = 8
HPC = H // NCORE     # heads per core = 2
TCH = 256            # phase-1 token chunk
NCH = T // TCH       # 16 chunks
NQ = 512             # q tile (free dim)
NKT = 16             # k tiles per batch (S / P)
ET = E // P          # 16 e-tiles
MASK_VAL = -1e5
SCALE = 1.0 / float(np.sqrt(DH))

_prog_cache = {}


def _build_program():
    import concourse.bass as bass
    import concourse.mybir as mybir
    import concourse.tile as tile
    from concourse import bacc

    f32 = mybir.dt.float32
    f32r = mybir.dt.float32r

    nc = bacc.Bacc("TRN2", target_bir_lowering=False, debug=False,
                   num_devices=NCORE)

    xT = nc.dram_tensor("xT", [E, T], f32r, kind="ExternalInput").ap()
    wqk = nc.dram_tensor("wqk", [E, 4 * P], f32r, kind="ExternalInput").ap()
    wv = nc.dram_tensor("wv", [E, 2 * P], f32r, kind="ExternalInput").ap()
    wout = nc.dram_tensor("wout", [E, E], f32r, kind="ExternalInput").ap()
    cs = nc.dram_tensor("cs", [2 * ROT, T], f32, kind="ExternalInput").ap()
    dmask = nc.dram_tensor("dmask", [4, P, NQ], f32, kind="ExternalInput").ap()
    ones = nc.dram_tensor("ones", [P, 1], f32r, kind="ExternalInput").ap()
    rmat = nc.dram_tensor("rmat", [P, ROT], f32r, kind="ExternalInput").ap()
    out = nc.dram_tensor("out", [E, T // NCORE], f32, kind="ExternalOutput").ap()

    with tile.TileContext(nc) as tc:
        with (
            tc.tile_pool(name="const", bufs=1) as const,
            tc.tile_pool(name="xcp", bufs=2) as xcp,
            tc.tile_pool(name="stg", bufs=8) as stg,
            tc.tile_pool(name="attn_in", bufs=1) as attn_in,
            tc.tile_pool(name="work", bufs=4) as work,
            tc.tile_pool(name="p4", bufs=2) as p4,
            tc.tile_pool(name="psum", bufs=2, space="PSUM") as psum,
            tc.tile_pool(name="dram", bufs=1, space="DRAM") as dram,
        ):
            # ---------------- resident constants ----------------
            wqk_sb = const.tile([P, ET, 4 * P], f32r, name="wqk_sb")
            nc.sync.dma_start(out=wqk_sb[:], in_=wqk.rearrange("(k p) f -> p k f", p=P))
            wv_sb = const.tile([P, ET, 2 * P], f32r, name="wv_sb")
            nc.sync.dma_start(out=wv_sb[:], in_=wv.rearrange("(k p) f -> p k f", p=P))
            cs_sb = const.tile([2 * ROT, T], f32, name="cs_sb")
            nc.sync.dma_start(out=cs_sb[:], in_=cs[:])
            dmask_sb = const.tile([P, 4, NQ], f32, name="dmask_sb")
            nc.sync.dma_start(out=dmask_sb[:], in_=dmask.rearrange("d p n -> p d n"))
            ones_sb = const.tile([P, 1], f32r, name="ones_sb")
            nc.sync.dma_start(out=ones_sb[:], in_=ones[:])
            rmat_sb = const.tile([P, ROT], f32r, name="rmat_sb")
            nc.sync.dma_start(out=rmat_sb[:], in_=rmat[:])

            # ---------------- DRAM intermediates ----------------
            qT_d = dram.tile([HPC, P, T], f32r, name="qT_d")
            kT_d = dram.tile([HPC, P, T], f32r, name="kT_d")
            v_d = dram.tile([T, HPC * P], f32r, name="v_d")
            a2a_in = dram.tile([NCORE, HPC * P, NQ], f32r, name="a2a_in")
            a2a_out = dram.tile([NCORE, HPC * P, NQ], f32r, name="a2a_out")

            xT_r = xT.rearrange("(k p) t -> p k t", p=P)

            # ---------------- phase 1: QKV projection + RoPE ----------------
            for tch in range(NCH):
                t0 = tch * TCH
                xc = xcp.tile([P, ET, TCH], f32r, name="xc")
                nc.sync.dma_start(out=xc[:], in_=xT_r[:, :, t0:t0 + TCH])

                # q/k f-tiles: [q_h0 | q_h1 | k_h0 | k_h1]
                sjs = []
                for j in range(4):
                    pq = psum.tile([P, TCH], f32, name="pq", tag="pA")
                    for et in range(ET):
                        nc.tensor.matmul(
                            pq[:],
                            wqk_sb[:, et, j * P:(j + 1) * P],
                            xc[:, et, :],
                            start=(et == 0), stop=(et == ET - 1),
                        )
                    sj = stg.tile([P, TCH], f32r, name="sj", tag="qs")
                    nc.scalar.copy(sj[:], pq[:])
                    sjs.append(sj)

                # RoPE on first ROT dims of each of the 4 tiles
                for j in range(4):
                    sj = sjs[j]
                    pr = psum.tile([ROT, TCH], f32, name="pr", tag="pC")
                    nc.tensor.matmul(pr[:], rmat_sb[:], sj[:],
                                     start=True, stop=True)
                    t1 = work.tile([ROT, TCH], f32, name="t1", tag="t1")
                    nc.vector.tensor_mul(t1[:], pr[:], cs_sb[ROT:, t0:t0 + TCH])
                    t2 = work.tile([ROT, TCH], f32, name="t2", tag="t2")
                    nc.vector.tensor_mul(t2[:], sj[:ROT, :], cs_sb[:ROT, t0:t0 + TCH])
                    nc.vector.tensor_add(sj[:ROT, :], t1[:], t2[:])

                for hl in range(HPC):
                    nc.sync.dma_start(out=qT_d[hl][:, t0:t0 + TCH], in_=sjs[hl][:])
                    nc.sync.dma_start(out=kT_d[hl][:, t0:t0 + TCH], in_=sjs[2 + hl][:])

                # V in token-major: out[t, dh] for both heads at once
                for ts4 in range(TCH // P):
                    pv = psum.tile([P, 2 * P], f32, name="pv", tag="pB")
                    for et in range(ET):
                        nc.tensor.matmul(
                            pv[:],
                            xc[:, et, ts4 * P:(ts4 + 1) * P],
                            wv_sb[:, et, :],
                            start=(et == 0), stop=(et == ET - 1),
                        )
                    vs = stg.tile([P, 2 * P], f32r, name="vs", tag="vs")
                    nc.scalar.copy(vs[:], pv[:])
                    nc.sync.dma_start(out=v_d[t0 + ts4 * P:t0 + (ts4 + 1) * P, :],
                                      in_=vs[:])

            # ---------------- phase 2: causal attention per (b, head) ----------------
            for b in range(B):
                for hl in range(HPC):
                    tb = b * S
                    kT_sb = attn_in.tile([P, S], f32r, name="kT_sb", tag="kT")
                    nc.sync.dma_start(out=kT_sb[:], in_=kT_d[hl][:, tb:tb + S])
                    qT_sb = attn_in.tile([P, S], f32r, name="qT_sb", tag="qT")
                    nc.sync.dma_start(out=qT_sb[:], in_=qT_d[hl][:, tb:tb + S])
                    v_sb = attn_in.tile([P, NKT, P], f32r, name="v_sb", tag="vA")
                    nc.sync.dma_start(
                        out=v_sb[:],
                        in_=v_d[tb:tb + S, hl * P:(hl + 1) * P]
                            .rearrange("(kt p) d -> p kt d", p=P))

                    for qt in range(S // NQ):
                        nkt = (qt + 1) * (NQ // P)
                        outp = psum.tile([P, NQ], f32, name="outp", tag="pB")
                        den = psum.tile([1, NQ], f32, name="den", tag="pC")
                        for kt in range(nkt):
                            sc = psum.tile([P, NQ], f32, name="sc", tag="pA")
                            nc.tensor.matmul(
                                sc[:],
                                kT_sb[:, kt * P:(kt + 1) * P],
                                qT_sb[:, qt * NQ:(qt + 1) * NQ],
                                start=True, stop=True)
                            d = kt - qt * (NQ // P)
                            if d >= 0:
                                nc.vector.tensor_add(sc[:], sc[:], dmask_sb[:, d, :])
                            at = work.tile([P, NQ], f32r, name="at", tag="at")
                            nc.scalar.activation(
                                at[:], sc[:], mybir.ActivationFunctionType.Exp,
                                scale=SCALE)
                            nc.tensor.matmul(den[:], ones_sb[:], at[:],
                                             start=(kt == 0), stop=(kt == nkt - 1))
                            nc.tensor.matmul(outp[:], v_sb[:, kt, :], at[:],
                                             start=(kt == 0), stop=(kt == nkt - 1))
                        rec = work.tile([1, NQ], f32, name="rec", tag="rec")
                        nc.vector.reciprocal(rec[:], den[:])
                        bc = work.tile([P, NQ], f32, name="bc", tag="bc")
                        nc.gpsimd.partition_broadcast(bc[:], rec[:])
                        nrm = work.tile([P, NQ], f32r, name="nrm", tag="nrm")
                        nc.vector.tensor_mul(nrm[:], outp[:], bc[:])
                        j = b * (S // NQ) + qt
                        nc.sync.dma_start(
                            out=a2a_in[j, hl * P:(hl + 1) * P, :], in_=nrm[:])

            # ---------------- phase 3: AllToAll head-slices -> token-slices ----------------
            nc.gpsimd.collective_compute(
                "AllToAll",
                mybir.AluOpType.bypass,
                replica_groups=[list(range(NCORE))],
                ins=[a2a_in[:]],
                outs=[a2a_out[:]],
            )

            # ---------------- phase 4: output projection for this core's 512 tokens ----------------
            fe_flat = a2a_out.rearrange("c e n -> (c e) n")
            wout_r = wout.rearrange("(k p) f -> p k f", p=P)
            for hp in range(2):
                outps = []
                for fi in range(8):
                    tag = "p" + "ABCD"[fi // 2]
                    po = psum.tile([P, NQ], f32, name=f"po{fi}", tag=tag)
                    outps.append(po)
                for et in range(ET):
                    fe = p4.tile([P, NQ], f32r, name="fe", tag="fe", bufs=4)
                    nc.sync.dma_start(out=fe[:], in_=fe_flat[et * P:(et + 1) * P, :])
                    wo = p4.tile([P, 8 * P], f32r, name="wo", tag="wo")
                    nc.sync.dma_start(out=wo[:],
                                      in_=wout_r[:, et, hp * 1024:hp * 1024 + 1024])
                    for fi in range(8):
                        nc.tensor.matmul(outps[fi][:],
                                         wo[:, fi * P:(fi + 1) * P], fe[:],
                                         start=(et == 0), stop=(et == ET - 1))
                for fi in range(8):
                    osb = p4.tile([P, NQ], f32, name="osb", tag="osb")
                    nc.scalar.copy(osb[:], outps[fi][:])
                    f0 = hp * 1024 + fi * P
                    nc.sync.dma_start(out=out[f0:f0 + P, :], in_=osb[:])

    nc.finalize()
    return nc


def _host_inputs(hidden_states, W_qkv, W_out):
    hidden_states = np.asarray(hidden_states, dtype=np.float32)
    W_qkv = np.asarray(W_qkv, dtype=np.float32)
    W_out = np.asarray(W_out, dtype=np.float32)

    xT = np.ascontiguousarray(hidden_states.reshape(T, E).T)

    # row indices into W_qkv for (q|k|v, head h, dh)
    def qrow(h):
        return (h // 2) * 3 * (E // MP) + (h % 2) * DH

    dh = np.arange(DH)
    wqk_list, wv_list = [], []
    for c in range(NCORE):
        cols = []
        for which in range(2):            # q then k
            for hl in range(HPC):
                h = HPC * c + hl
                rows = qrow(h) + which * (E // MP) + dh
                cols.append(W_qkv[rows, :])
        wqk_c = np.concatenate(cols, axis=0).T          # [E, 4*128]
        wqk_list.append(np.ascontiguousarray(wqk_c))
        vcols = []
        for hl in range(HPC):
            h = HPC * c + hl
            rows = qrow(h) + 2 * (E // MP) + dh
            vcols.append(W_qkv[rows, :])
        wv_c = np.concatenate(vcols, axis=0).T          # [E, 2*128]
        wv_list.append(np.ascontiguousarray(wv_c))

    wout_T = np.ascontiguousarray(W_out.T)              # [E, E]

    # rope tables: cs[d, t] = cos(ang(t mod S, d//2)), rows 32.. = sin
    inv_freq = 1.0 / (10000.0 ** (np.arange(0, ROT, 2, dtype=np.float64) / ROT))
    ang = np.arange(S, dtype=np.float64)[:, None] * inv_freq[None, :]   # [S, 16]
    cos2 = np.repeat(np.cos(ang), 2, axis=1).T                          # [32, S]
    sin2 = np.repeat(np.sin(ang), 2, axis=1).T
    cs = np.concatenate([np.tile(cos2, (1, B)), np.tile(sin2, (1, B))],
                        axis=0).astype(np.float32)                      # [64, T]

    kk = np.arange(P)[:, None]
    qq = np.arange(NQ)[None, :]
    dmask = np.stack(
        [np.where(P * d + kk <= qq, 0.0, MASK_VAL) for d in range(4)]
    ).astype(np.float32)                                                # [4,128,512]

    ones = np.ones((P, 1), dtype=np.float32)

    rmat = np.zeros((P, ROT), dtype=np.float32)
    for m in range(ROT):
        if m % 2 == 0:
            rmat[m + 1, m] = -1.0
        else:
            rmat[m - 1, m] = 1.0

    in_maps = []
    for c in range(NCORE):
        in_maps.append({
            "xT": xT,
            "wqk": wqk_list[c],
            "wv": wv_list[c],
            "wout": wout_T,
            "cs": cs,
            "dmask": dmask,
            "ones": ones,
            "rmat": rmat,
        })
    return in_maps


def kernel(hidden_states, W_qkv, W_out):
    from concourse.bass_utils import run_bass_kernel_spmd

    if "nc" not in _prog_cache:
        _prog_cache["nc"] = _build_program()
    nc = _prog_cache["nc"]

    in_maps = _host_inputs(hidden_states, W_qkv, W_out)
    import os
    trace = bool(int(os.environ.get("KERNEL_TRACE", "0")))
    res = run_bass_kernel_spmd(nc, in_maps, list(range(NCORE)), trace=trace)
    _prog_cache["last_result"] = res

    full = np.empty((B, S, E), dtype=np.float32)
    for c in range(NCORE):
        oc = res.results[c]["out"]          # [E, 512] token-slice, f-major
        b = c // (S // NQ)
        s0 = (c % (S // NQ)) * NQ
        full[b, s0:s0 + NQ, :] = oc.T
    return full
